# revision 5
# baseline (speedup 1.0000x reference)
"""DeepFM forward kernel for Trainium2, data-parallel over 8 NeuronCores.

Math refactor vs the straightforward DeepFM graph:
  1. The 256-wide tower dense outputs are only consumed by (a) the FM
     interaction sum and (b) MLP layer 1. (a) collapses to 16 "fold" sums
     per tower (sum_ij m_i.u_j = sum_k (sum_i m_ik)(sum_j u_jk)) and (b) is
     linear, so W1 is folded into the tower weights host-side:
         z1 = xm @ (Wm_d @ W1[:256]) + xu @ (Wu_d @ W1[256:]) + b1'
     The dense tower outputs are never materialized on chip.
  2. The FM sum uses the polarization identity
         sum_k fold_m.fold_u + add = sum_k (p_k^2 - q_k^2)/4 + a
     with p = fold_m + fold_u, q = fold_m - fold_u (both linear in x), and
     the additive term a riding rows 32/33 as ((a+1)/2)^2 - ((a-1)/2)^2 = a.
     The whole FM side is ONE 34-row matmul accumulation chain plus ONE
     scalar-engine Square activation, folding into the final matmul.

Everything runs in bf16 (fp32 PSUM accumulate): halves DMA traffic and
enables FWL fast weight loads. Inputs are repacked host-side so each batch
tile (both towers) is one fully-contiguous 8KB-per-partition DMA; tile 0 is
split in two so the first matmuls can start earliest. The two final M=1
matmuls are col-tiled onto PE column strip 2 (tile_position=(0,64)) so they
run concurrently with the next tile's M=34 extras chain on strips 0-1.
"""

import numpy as np
import ml_dtypes

import concourse.bacc as bacc
import concourse.bass as bass  # noqa: F401
import concourse.mybir as mybir
import concourse.tile as tile
from concourse.bass_utils import run_bass_kernel_spmd

N_CORES = 8
B_FULL = 16384
R = B_FULL // N_CORES  # 2048 rows per core
F = 512                # input features per tower
KC = F // 128          # 4 contraction chunks per tower
NT = 512               # batch tile on the free dim
NTILES = R // NT       # 4
NX = 34                # extras rows: p(16) + q(16) + a-rows(2)
N_WARM = 4             # PE pre-warm matmuls
XT_COLS = 2 * KC * NT  # per-tile input cols (both towers)

F32 = mybir.dt.float32
BF16 = mybir.dt.bfloat16

# bf16 weight-pack column offsets ([128, WCOLS] blob)
Z1_OFF = 0                      # 16 x 128: (g, j) at (g*8+j)*128; j=0-3 Am, 4-7 Au
Z1_COLS = 16 * 128
X_OFF = Z1_COLS                 # 8 x 34: j=0-3 Xm chunks, 4-7 Xu chunks
W2_OFF = X_OFF + 8 * NX         # 2 x 128
W3_OFF = W2_OFF + 2 * 128       # 1
WQ_OFF = W3_OFF + 1             # 1 (rows 0-33 = [1/4]*16 + [-1/4]*16 + [1, -1])
WCOLS = WQ_OFF + 1

# fp32 bias-pack column indices ([128, BCOLS])
B1A, B1B, BX, B2C = range(4)
BCOLS = 4

FIN_POS = 64  # PE column strip for the col-tiled final matmuls


def _chunk(Wext):
    """[K, M] -> [128, (K/128)*M]: K-chunk k occupies cols [k*M, (k+1)*M)."""
    kc, m = Wext.shape[0] // 128, Wext.shape[1]
    return Wext.reshape(kc, 128, m).transpose(1, 0, 2).reshape(128, kc * m)


def _col(vec):
    out = np.zeros((128, 1), np.float32)
    out[: len(vec), 0] = vec
    return out


def _pack_weights(Wm, bm, Wu, bu, W1, b1, W2, b2, W3, b3):
    f64 = np.float64
    Wm, bm, Wu, bu = Wm.astype(f64), bm.astype(f64), Wu.astype(f64), bu.astype(f64)
    W1, b1, W2, b2 = W1.astype(f64), b1.astype(f64), W2.astype(f64), b2.astype(f64)
    b3v = float(np.asarray(b3, f64).reshape(-1)[0])

    # fused z1 = xm @ Am + xu @ Au + b1p
    Am = Wm[:, :256] @ W1[:256, :]
    Au = Wu[:, :256] @ W1[256:, :]
    b1p = b1 + bm[:256] @ W1[:256, :] + bu[:256] @ W1[256:, :]

    # FM extras: p/q fold rows + additive rows
    FWm = Wm[:, :256].reshape(F, 16, 16).sum(axis=1)  # [512, 16]
    FWu = Wu[:, :256].reshape(F, 16, 16).sum(axis=1)
    fbm = bm[:256].reshape(16, 16).sum(axis=0)
    fbu = bu[:256].reshape(16, 16).sum(axis=0)
    awm, awu = Wm[:, 256], Wu[:, 256]
    A = bm[256] + bu[256] + b3v
    Xm = np.concatenate([FWm, FWm, awm[:, None] / 2, awm[:, None] / 2], axis=1)
    Xu = np.concatenate([FWu, -FWu, awu[:, None] / 2, awu[:, None] / 2], axis=1)
    xbias = np.concatenate([fbm + fbu, fbm - fbu, [(A + 1) / 2], [(A - 1) / 2]])
    wq = np.concatenate([np.full(16, 0.25), np.full(16, -0.25), [1.0, -1.0]])

    # z1 block in accumulation order: group g, then xm chunks 0-3, xu chunks 0-3
    amc, auc = _chunk(Am), _chunk(Au)  # [128, 4*256]
    z1_cols = []
    for g in range(2):
        for k in range(KC):
            z1_cols.append(amc[:, k * 256 + g * 128 : k * 256 + (g + 1) * 128])
        for k in range(KC):
            z1_cols.append(auc[:, k * 256 + g * 128 : k * 256 + (g + 1) * 128])
    wq_col = np.zeros((128, 1), f64)
    wq_col[:NX, 0] = wq
    wp = np.concatenate(
        z1_cols
        + [_chunk(Xm), _chunk(Xu), _chunk(W2), np.asarray(W3, f64).reshape(128, 1), wq_col],
        axis=1,
    )
    assert wp.shape == (128, WCOLS), wp.shape
    bp = np.concatenate(
        [_col(b1p[:128]), _col(b1p[128:]), _col(xbias), _col(b2)], axis=1
    )
    return (
        np.ascontiguousarray(wp.astype(ml_dtypes.bfloat16)),
        np.ascontiguousarray(bp.astype(np.float32)),
    )


def _build_bass():
    nc = bacc.Bacc()
    x = nc.dram_tensor("x", [128, NTILES * XT_COLS], BF16, kind="ExternalInput")
    wp = nc.dram_tensor("wp", [128, WCOLS], BF16, kind="ExternalInput")
    bp = nc.dram_tensor("bp", [128, BCOLS], F32, kind="ExternalInput")
    out = nc.dram_tensor("out", [1, R], F32, kind="ExternalOutput")

    relu = mybir.ActivationFunctionType.Relu
    square = mybir.ActivationFunctionType.Square

    with tile.TileContext(nc) as tc:
        with (
            tc.tile_pool(name="wpool", bufs=1) as wpool,
            tc.tile_pool(name="xpool", bufs=1) as xpool,
            tc.tile_pool(name="dpool", bufs=1) as dpool,
            tc.tile_pool(name="opool", bufs=1) as opool,
            tc.tile_pool(name="psz", bufs=4, space="PSUM") as psz,
            tc.tile_pool(name="psx", bufs=2, space="PSUM") as psx,
            tc.tile_pool(name="psm", bufs=1, space="PSUM") as psm,
            tc.tile_pool(name="psf", bufs=1, space="PSUM") as psf,
        ):
            # PE pre-warm on a zeroed tile, pipelined across the 4 z1 PSUM
            # banks: covers the HAM activity window until real data lands so
            # real matmuls run at 2.4 GHz.
            wgar = wpool.tile([128, NT], BF16)
            nc.vector.memset(wgar, 0.0)
            for _ in range(N_WARM):
                pw = psz.tile([128, NT], F32, name="ps_z1")
                nc.tensor.matmul(pw, wgar[:, :128], wgar, start=True, stop=True)

            # Weights on the scalar HWDGE ring (inputs get the sync ring),
            # in need-order: z1 group 0, z1 group 1, then the rest.
            w = wpool.tile([128, WCOLS], BF16)
            nc.scalar.dma_start(out=w[:, : Z1_COLS // 2], in_=wp[:, : Z1_COLS // 2])
            nc.scalar.dma_start(
                out=w[:, Z1_COLS // 2 : Z1_COLS], in_=wp[:, Z1_COLS // 2 : Z1_COLS]
            )
            b = wpool.tile([128, BCOLS], F32)
            nc.scalar.dma_start(out=w[:, Z1_COLS:], in_=wp[:, Z1_COLS:])
            nc.scalar.dma_start(out=b, in_=bp[:, :])
            out_sb = opool.tile([128, NT], F32)

            xr = x.rearrange(
                "p (t w c n) -> p t w c n", t=NTILES, w=2, c=KC, n=NT
            )

            # Input tiles prefetch up front, one contiguous DMA per tile
            # (8KB/partition); tile 0 split per tower so compute starts early.
            xts = []
            for t in range(NTILES):
                xt = xpool.tile([128, 2, KC, NT], BF16, name=f"x{t}")
                if t == 0:
                    nc.sync.dma_start(out=xt[:, 0], in_=xr[:, t, 0])
                    nc.sync.dma_start(out=xt[:, 1], in_=xr[:, t, 1])
                else:
                    nc.sync.dma_start(out=xt, in_=xr[:, t])
                xts.append(xt)

            h1s, sqs, h2s = {}, {}, {}

            def emit_z1(t):
                xt = xts[t]
                for g in range(2):
                    ps = psz.tile([128, NT], F32, name="ps_z1")
                    for j in range(8):
                        tw, k = (0, j) if j < 4 else (1, j - 4)
                        lhsT = w[:, (g * 8 + j) * 128 : (g * 8 + j + 1) * 128]
                        nc.tensor.matmul(
                            ps, lhsT, xt[:, tw, k, :], start=(j == 0), stop=(j == 7)
                        )
                    h = dpool.tile([128, NT], BF16, name=f"h1_{t}_{g}")
                    nc.scalar.activation(
                        out=h, in_=ps, func=relu, bias=b[:, g : g + 1]
                    )
                    h1s[(t, g)] = h

            def emit_extras(t, mid=None):
                xt = xts[t]
                ps = psx.tile([NX, NT], F32, name="ps_x")
                for j in range(8):
                    tw, k = (0, j) if j < 4 else (1, j - 4)
                    lhsT = w[:, X_OFF + j * NX : X_OFF + (j + 1) * NX]
                    nc.tensor.matmul(
                        ps, lhsT, xt[:, tw, k, :], start=(j == 0), stop=(j == 7)
                    )
                    if j == 0 and mid is not None:
                        mid()
                sq = dpool.tile([NX, NT], BF16, name=f"sq_{t}")
                nc.scalar.activation(
                    out=sq, in_=ps, func=square, bias=b[:NX, BX : BX + 1]
                )
                sqs[t] = sq

            def emit_mlp2(t):
                ps = psm.tile([128, NT], F32, name="ps_m")
                for c in range(2):
                    lhsT = w[:, W2_OFF + c * 128 : W2_OFF + (c + 1) * 128]
                    nc.tensor.matmul(
                        ps, lhsT, h1s[(t, c)], start=(c == 0), stop=(c == 1)
                    )
                h2 = dpool.tile([128, NT], BF16, name=f"h2_{t}")
                nc.scalar.activation(
                    out=h2, in_=ps, func=relu, bias=b[:, B2C : B2C + 1]
                )
                h2s[t] = h2

            def emit_final(t):
                # M=1 matmuls col-tiled onto strip 2: run concurrently with
                # the extras chain (strips 0-1) of the next tile.
                ps = psf.tile([128, NT], F32, name="ps_f")
                pf = ps[FIN_POS : FIN_POS + 1]
                nc.tensor.matmul(
                    pf, w[:, W3_OFF : W3_OFF + 1], h2s[t],
                    start=True, stop=False, tile_position=(0, FIN_POS),
                )
                nc.tensor.matmul(
                    pf, w[:NX, WQ_OFF : WQ_OFF + 1], sqs[t],
                    start=False, stop=True, tile_position=(0, FIN_POS),
                )
                n0 = t * NT
                ob = out_sb[FIN_POS : FIN_POS + 1, :]
                nc.vector.tensor_copy(ob, pf)
                nc.scalar.dma_start(out=out[:, n0 : n0 + NT], in_=ob)

            for t in range(NTILES):
                emit_z1(t)
                mid = (lambda tt=t: emit_final(tt - 1)) if t > 0 else None
                emit_extras(t, mid=mid)
                emit_mlp2(t)
            emit_final(NTILES - 1)
    nc.finalize()
    return nc


def _pack_x(xmT_core, xuT_core):
    """2x [512, 2048] fp32 -> [128, NTILES*2*KC*NT] bf16, tile-contiguous."""
    ym = xmT_core.reshape(KC, 128, NTILES, NT).transpose(1, 2, 0, 3)  # [p,t,c,n]
    yu = xuT_core.reshape(KC, 128, NTILES, NT).transpose(1, 2, 0, 3)
    y = np.stack([ym, yu], axis=2)  # [p, t, w, c, n]
    return np.ascontiguousarray(
        y.reshape(128, NTILES * XT_COLS).astype(ml_dtypes.bfloat16)
    )


_NC_CACHE = []


def kernel(movie_vectors, user_vectors, Wm, bm, Wu, bu, W1, b1, W2, b2, W3, b3):
    movie_vectors = np.asarray(movie_vectors, np.float32)
    user_vectors = np.asarray(user_vectors, np.float32)
    wp, bp = _pack_weights(
        np.asarray(Wm, np.float32), np.asarray(bm, np.float32),
        np.asarray(Wu, np.float32), np.asarray(bu, np.float32),
        np.asarray(W1, np.float32), np.asarray(b1, np.float32),
        np.asarray(W2, np.float32), np.asarray(b2, np.float32),
        np.asarray(W3, np.float32), np.asarray(b3, np.float32),
    )
    xmT = movie_vectors.T  # [512, 16384]
    xuT = user_vectors.T

    if not _NC_CACHE:
        _NC_CACHE.append(_build_bass())
    nc = _NC_CACHE[0]

    in_maps = []
    for c in range(N_CORES):
        sl = slice(c * R, (c + 1) * R)
        in_maps.append(
            {"x": _pack_x(xmT[:, sl], xuT[:, sl]), "wp": wp, "bp": bp}
        )
    res = run_bass_kernel_spmd(nc, in_maps, core_ids=list(range(N_CORES)))
    kernel.last_result = res
    return np.concatenate([r["out"].reshape(R, 1) for r in res.results], axis=0)


# revision 9
# speedup vs baseline: 1.0015x; 1.0015x over previous
"""DeepFM forward kernel for Trainium2, data-parallel over 8 NeuronCores.

Math refactor vs the straightforward DeepFM graph:
  1. The 256-wide tower dense outputs are only consumed by (a) the FM
     interaction sum and (b) MLP layer 1. (a) collapses to 16 "fold" sums
     per tower (sum_ij m_i.u_j = sum_k (sum_i m_ik)(sum_j u_jk)) and (b) is
     linear, so W1 is folded into the tower weights host-side:
         z1 = xm @ (Wm_d @ W1[:256]) + xu @ (Wu_d @ W1[256:]) + b1'
     The dense tower outputs are never materialized on chip.
  2. The FM sum uses the polarization identity
         sum_k fold_m.fold_u + add = sum_k (p_k^2 - q_k^2)/4 + a
     with p = fold_m + fold_u, q = fold_m - fold_u (both linear in x), and
     the additive term a riding rows 32/33 as ((a+1)/2)^2 - ((a-1)/2)^2 = a.
     The whole FM side is ONE 34-row matmul accumulation chain plus ONE
     scalar-engine Square activation, folding into the final matmul.

Everything runs in bf16 (fp32 PSUM accumulate): halves DMA traffic and
enables FWL fast weight loads. Inputs are repacked host-side so each batch
tile (both towers) is one fully-contiguous 8KB-per-partition DMA; tile 0 is
split in two so the first matmuls can start earliest. The two final M=1
matmuls are col-tiled onto PE column strip 2 (tile_position=(0,64)) so they
run concurrently with the next tile's M=34 extras chain on strips 0-1.
"""

import numpy as np
import ml_dtypes

import concourse.bacc as bacc
import concourse.bass as bass  # noqa: F401
import concourse.mybir as mybir
import concourse.tile as tile
from concourse.bass_utils import run_bass_kernel_spmd

N_CORES = 8
B_FULL = 16384
R = B_FULL // N_CORES  # 2048 rows per core
F = 512                # input features per tower
KC = F // 128          # 4 contraction chunks per tower
NT = 512               # batch tile on the free dim
NTILES = R // NT       # 4
NX = 34                # extras rows: p(16) + q(16) + a-rows(2)
N_WARM = 6             # PE pre-warm matmuls
XT_COLS = 2 * KC * NT  # per-tile input cols (both towers)

F32 = mybir.dt.float32
BF16 = mybir.dt.bfloat16

# bf16 weight-pack column offsets ([128, WCOLS] blob)
Z1_OFF = 0                      # 16 x 128: (g, j) at (g*8+j)*128; j=0-3 Am, 4-7 Au
Z1_COLS = 16 * 128
X_OFF = Z1_COLS                 # 8 x 34: j=0-3 Xm chunks, 4-7 Xu chunks
W2_OFF = X_OFF + 8 * NX         # 2 x 128
W3_OFF = W2_OFF + 2 * 128       # 1
WQ_OFF = W3_OFF + 1             # 1 (rows 0-33 = [1/4]*16 + [-1/4]*16 + [1, -1])
WCOLS = WQ_OFF + 1

# fp32 bias-pack column indices ([128, BCOLS])
B1A, B1B, BX, B2C = range(4)
BCOLS = 4

FIN_POS = 64  # PE column strip for the col-tiled final matmuls


def _chunk(Wext):
    """[K, M] -> [128, (K/128)*M]: K-chunk k occupies cols [k*M, (k+1)*M)."""
    kc, m = Wext.shape[0] // 128, Wext.shape[1]
    return Wext.reshape(kc, 128, m).transpose(1, 0, 2).reshape(128, kc * m)


def _col(vec):
    out = np.zeros((128, 1), np.float32)
    out[: len(vec), 0] = vec
    return out


def _pack_weights(Wm, bm, Wu, bu, W1, b1, W2, b2, W3, b3):
    f64 = np.float64
    Wm, bm, Wu, bu = Wm.astype(f64), bm.astype(f64), Wu.astype(f64), bu.astype(f64)
    W1, b1, W2, b2 = W1.astype(f64), b1.astype(f64), W2.astype(f64), b2.astype(f64)
    b3v = float(np.asarray(b3, f64).reshape(-1)[0])

    # fused z1 = xm @ Am + xu @ Au + b1p
    Am = Wm[:, :256] @ W1[:256, :]
    Au = Wu[:, :256] @ W1[256:, :]
    b1p = b1 + bm[:256] @ W1[:256, :] + bu[:256] @ W1[256:, :]

    # FM extras: p/q fold rows + additive rows
    FWm = Wm[:, :256].reshape(F, 16, 16).sum(axis=1)  # [512, 16]
    FWu = Wu[:, :256].reshape(F, 16, 16).sum(axis=1)
    fbm = bm[:256].reshape(16, 16).sum(axis=0)
    fbu = bu[:256].reshape(16, 16).sum(axis=0)
    awm, awu = Wm[:, 256], Wu[:, 256]
    A = bm[256] + bu[256] + b3v
    Xm = np.concatenate([FWm, FWm, awm[:, None] / 2, awm[:, None] / 2], axis=1)
    Xu = np.concatenate([FWu, -FWu, awu[:, None] / 2, awu[:, None] / 2], axis=1)
    xbias = np.concatenate([fbm + fbu, fbm - fbu, [(A + 1) / 2], [(A - 1) / 2]])
    wq = np.concatenate([np.full(16, 0.25), np.full(16, -0.25), [1.0, -1.0]])

    # z1 block in accumulation order: group g, then xm chunks 0-3, xu chunks 0-3
    amc, auc = _chunk(Am), _chunk(Au)  # [128, 4*256]
    z1_cols = []
    for g in range(2):
        for k in range(KC):
            z1_cols.append(amc[:, k * 256 + g * 128 : k * 256 + (g + 1) * 128])
        for k in range(KC):
            z1_cols.append(auc[:, k * 256 + g * 128 : k * 256 + (g + 1) * 128])
    wq_col = np.zeros((128, 1), f64)
    wq_col[:NX, 0] = wq
    wp = np.concatenate(
        z1_cols
        + [_chunk(Xm), _chunk(Xu), _chunk(W2), np.asarray(W3, f64).reshape(128, 1), wq_col],
        axis=1,
    )
    assert wp.shape == (128, WCOLS), wp.shape
    bp = np.concatenate(
        [_col(b1p[:128]), _col(b1p[128:]), _col(xbias), _col(b2)], axis=1
    )
    return (
        np.ascontiguousarray(wp.astype(ml_dtypes.bfloat16)),
        np.ascontiguousarray(bp.astype(np.float32)),
    )


def _build_bass():
    nc = bacc.Bacc()
    x = nc.dram_tensor("x", [128, NTILES * XT_COLS], BF16, kind="ExternalInput")
    wp = nc.dram_tensor("wp", [128, WCOLS], BF16, kind="ExternalInput")
    bp = nc.dram_tensor("bp", [128, BCOLS], F32, kind="ExternalInput")
    out = nc.dram_tensor("out", [1, R], F32, kind="ExternalOutput")

    relu = mybir.ActivationFunctionType.Relu
    square = mybir.ActivationFunctionType.Square

    with tile.TileContext(nc) as tc:
        with (
            tc.tile_pool(name="wpool", bufs=1) as wpool,
            tc.tile_pool(name="xpool", bufs=1) as xpool,
            tc.tile_pool(name="dpool", bufs=1) as dpool,
            tc.tile_pool(name="opool", bufs=1) as opool,
            tc.tile_pool(name="psz", bufs=4, space="PSUM") as psz,
            tc.tile_pool(name="psx", bufs=2, space="PSUM") as psx,
            tc.tile_pool(name="psm", bufs=1, space="PSUM") as psm,
            tc.tile_pool(name="psf", bufs=1, space="PSUM") as psf,
        ):
            # PE pre-warm matmuls on a zeroed tile, pipelined across the z1
            # PSUM banks: covers the HAM activity window until real data
            # lands so real matmuls run at 2.4 GHz.
            wgar = wpool.tile([128, NT], BF16)
            nc.vector.memset(wgar, 0.0)
            for _ in range(N_WARM):
                pw = psz.tile([128, NT], F32, name="ps_z1")
                nc.tensor.matmul(pw, wgar[:, :128], wgar, start=True, stop=True)

            # Weights on the scalar HWDGE ring (inputs get the sync ring),
            # in need-order: z1 group 0, z1 group 1, then the rest. Each
            # piece is its own tile so consumers wait only on their own DMA.
            w0 = wpool.tile([128, Z1_COLS // 2], BF16)
            nc.scalar.dma_start(out=w0, in_=wp[:, : Z1_COLS // 2])
            w1 = wpool.tile([128, Z1_COLS // 2], BF16)
            nc.scalar.dma_start(out=w1, in_=wp[:, Z1_COLS // 2 : Z1_COLS])
            wr = wpool.tile([128, WCOLS - Z1_COLS], BF16)
            nc.scalar.dma_start(out=wr, in_=wp[:, Z1_COLS:])
            b = wpool.tile([128, BCOLS], F32)
            nc.scalar.dma_start(out=b, in_=bp[:, :])
            wz = (w0, w1)
            out_sb = opool.tile([128, NT], F32)

            xr = x.rearrange(
                "p (t w c n) -> p t w c n", t=NTILES, w=2, c=KC, n=NT
            )

            # Input tiles prefetch up front, one contiguous DMA per tile
            # (8KB/partition); tile 0 split per tower (separate tiles) so the
            # first matmuls depend only on the xm half.
            x0m = xpool.tile([128, KC, NT], BF16)
            nc.sync.dma_start(out=x0m, in_=xr[:, 0, 0])
            x0u = xpool.tile([128, KC, NT], BF16)
            nc.sync.dma_start(out=x0u, in_=xr[:, 0, 1])
            xts = [(x0m, x0u)]
            for t in range(1, NTILES):
                xt = xpool.tile([128, 2, KC, NT], BF16, name=f"x{t}")
                nc.sync.dma_start(out=xt, in_=xr[:, t])
                xts.append((xt[:, 0], xt[:, 1]))

            h1s, sqs, h2s = {}, {}, {}

            def emit_z1(t):
                xmt, xut = xts[t]
                for g in range(2):
                    ps = psz.tile([128, NT], F32, name="ps_z1")
                    for j in range(8):
                        xt, k = (xmt, j) if j < 4 else (xut, j - 4)
                        lhsT = wz[g][:, j * 128 : (j + 1) * 128]
                        nc.tensor.matmul(
                            ps, lhsT, xt[:, k, :], start=(j == 0), stop=(j == 7)
                        )
                    h = dpool.tile([128, NT], BF16, name=f"h1_{t}_{g}")
                    nc.scalar.activation(
                        out=h, in_=ps, func=relu, bias=b[:, g : g + 1]
                    )
                    h1s[(t, g)] = h

            def emit_extras(t, mid=None):
                xmt, xut = xts[t]
                ps = psx.tile([NX, NT], F32, name="ps_x")
                for j in range(8):
                    xt, k = (xmt, j) if j < 4 else (xut, j - 4)
                    lhsT = wr[:, j * NX : (j + 1) * NX]
                    nc.tensor.matmul(
                        ps, lhsT, xt[:, k, :], start=(j == 0), stop=(j == 7)
                    )
                    if j == 0 and mid is not None:
                        mid()
                sq = dpool.tile([NX, NT], BF16, name=f"sq_{t}")
                nc.scalar.activation(
                    out=sq, in_=ps, func=square, bias=b[:NX, BX : BX + 1]
                )
                sqs[t] = sq

            def emit_mlp2(t):
                ps = psm.tile([128, NT], F32, name="ps_m")
                for c in range(2):
                    lhsT = wr[:, W2_OFF - X_OFF + c * 128 : W2_OFF - X_OFF + (c + 1) * 128]
                    nc.tensor.matmul(
                        ps, lhsT, h1s[(t, c)], start=(c == 0), stop=(c == 1)
                    )
                h2 = dpool.tile([128, NT], BF16, name=f"h2_{t}")
                nc.scalar.activation(
                    out=h2, in_=ps, func=relu, bias=b[:, B2C : B2C + 1]
                )
                h2s[t] = h2

            def emit_final(t):
                # M=1 matmuls col-tiled onto strip 2: run concurrently with
                # the extras chain (strips 0-1) of the next tile.
                ps = psf.tile([128, NT], F32, name="ps_f")
                pf = ps[FIN_POS : FIN_POS + 1]
                nc.tensor.matmul(
                    pf, wr[:, W3_OFF - X_OFF : W3_OFF - X_OFF + 1], h2s[t],
                    start=True, stop=False, tile_position=(0, FIN_POS),
                )
                nc.tensor.matmul(
                    pf, wr[:NX, WQ_OFF - X_OFF : WQ_OFF - X_OFF + 1], sqs[t],
                    start=False, stop=True, tile_position=(0, FIN_POS),
                )
                n0 = t * NT
                ob = out_sb[FIN_POS : FIN_POS + 1, :]
                nc.vector.tensor_copy(ob, pf)
                nc.scalar.dma_start(out=out[:, n0 : n0 + NT], in_=ob)

            for t in range(NTILES):
                emit_z1(t)
                mid = (lambda tt=t: emit_final(tt - 1)) if t > 0 else None
                emit_extras(t, mid=mid)
                emit_mlp2(t)
            emit_final(NTILES - 1)
    nc.finalize()
    return nc


def _pack_x(xmT_core, xuT_core):
    """2x [512, 2048] fp32 -> [128, NTILES*2*KC*NT] bf16, tile-contiguous."""
    ym = xmT_core.reshape(KC, 128, NTILES, NT).transpose(1, 2, 0, 3)  # [p,t,c,n]
    yu = xuT_core.reshape(KC, 128, NTILES, NT).transpose(1, 2, 0, 3)
    y = np.stack([ym, yu], axis=2)  # [p, t, w, c, n]
    return np.ascontiguousarray(
        y.reshape(128, NTILES * XT_COLS).astype(ml_dtypes.bfloat16)
    )


_NC_CACHE = []


def kernel(movie_vectors, user_vectors, Wm, bm, Wu, bu, W1, b1, W2, b2, W3, b3):
    movie_vectors = np.asarray(movie_vectors, np.float32)
    user_vectors = np.asarray(user_vectors, np.float32)
    wp, bp = _pack_weights(
        np.asarray(Wm, np.float32), np.asarray(bm, np.float32),
        np.asarray(Wu, np.float32), np.asarray(bu, np.float32),
        np.asarray(W1, np.float32), np.asarray(b1, np.float32),
        np.asarray(W2, np.float32), np.asarray(b2, np.float32),
        np.asarray(W3, np.float32), np.asarray(b3, np.float32),
    )
    xmT = movie_vectors.T  # [512, 16384]
    xuT = user_vectors.T

    if not _NC_CACHE:
        _NC_CACHE.append(_build_bass())
    nc = _NC_CACHE[0]

    in_maps = []
    for c in range(N_CORES):
        sl = slice(c * R, (c + 1) * R)
        in_maps.append(
            {"x": _pack_x(xmT[:, sl], xuT[:, sl]), "wp": wp, "bp": bp}
        )
    res = run_bass_kernel_spmd(nc, in_maps, core_ids=list(range(N_CORES)))
    kernel.last_result = res
    return np.concatenate([r["out"].reshape(R, 1) for r in res.results], axis=0)


# revision 17
# speedup vs baseline: 1.0365x; 1.0350x over previous
"""DeepFM forward kernel for Trainium2, data-parallel over 8 NeuronCores.

Math refactor vs the straightforward DeepFM graph:
  1. The 256-wide tower dense outputs are only consumed by (a) the FM
     interaction sum and (b) MLP layer 1. (a) collapses to 16 "fold" sums
     per tower (sum_ij m_i.u_j = sum_k (sum_i m_ik)(sum_j u_jk)) and (b) is
     linear, so W1 is folded into the tower weights host-side:
         z1 = xm @ (Wm_d @ W1[:256]) + xu @ (Wu_d @ W1[256:]) + b1'
     The dense tower outputs are never materialized on chip.
  2. The FM sum uses the polarization identity
         sum_k fold_m.fold_u + add = sum_k (p_k^2 - q_k^2)/4 + a
     with p = fold_m + fold_u, q = fold_m - fold_u (both linear in x), and
     the additive term a riding rows 32/33 as ((a+1)/2)^2 - ((a-1)/2)^2 = a.
     The whole FM side is ONE 34-row matmul accumulation chain plus ONE
     scalar-engine Square activation, folding into the final matmul.

Everything runs in bf16 (fp32 PSUM accumulate): halves DMA traffic and
enables FWL fast weight loads. Inputs are repacked host-side so each batch
tile (both towers) is one fully-contiguous 8KB-per-partition DMA; tile 0 is
split in two so the first matmuls can start earliest. The two final M=1
matmuls are col-tiled onto PE column strip 2 (tile_position=(0,64)) so they
run concurrently with the next tile's M=34 extras chain on strips 0-1.
"""

import numpy as np
import ml_dtypes

import concourse.bacc as bacc
import concourse.bass as bass  # noqa: F401
import concourse.mybir as mybir
import concourse.tile as tile
from concourse.bass_utils import run_bass_kernel_spmd

N_CORES = 8
B_FULL = 16384
R = B_FULL // N_CORES  # 2048 rows per core
F = 512                # input features per tower
KC = F // 128          # 4 contraction chunks per tower
NT = 512               # batch tile on the free dim
NTILES = R // NT       # 4
NX = 34                # extras rows: p(16) + q(16) + a-rows(2)
N_WARM = 5             # PE pre-warm matmuls
MPAD = 33              # final matmuls padded to M=33 so they share the
                       # 128x64 col-tiling mode with the extras chain
XT_COLS = 2 * KC * NT  # per-tile input cols (both towers)

F32 = mybir.dt.float32
BF16 = mybir.dt.bfloat16

# bf16 weight-pack column offsets ([128, WCOLS] blob)
Z1_OFF = 0                      # 16 x 128: (g, j) at (g*8+j)*128; j=0-3 Am, 4-7 Au
Z1_COLS = 16 * 128
X_OFF = Z1_COLS                 # 8 x 34: j=0-3 Xm chunks, 4-7 Xu chunks
W2_OFF = X_OFF + 8 * NX         # 2 x 128
W3_OFF = W2_OFF + 2 * 128       # MPAD cols (col 0 = W3, rest zero)
WQ_OFF = W3_OFF + MPAD          # MPAD cols (col 0 rows 0-33 = fm quad weights)
WCOLS = WQ_OFF + MPAD

# fp32 bias-pack column indices ([128, BCOLS])
B1A, B1B, BX, B2C = range(4)
BCOLS = 4

FIN_POS = 64  # PE column strip for the col-tiled final matmuls


def _chunk(Wext):
    """[K, M] -> [128, (K/128)*M]: K-chunk k occupies cols [k*M, (k+1)*M)."""
    kc, m = Wext.shape[0] // 128, Wext.shape[1]
    return Wext.reshape(kc, 128, m).transpose(1, 0, 2).reshape(128, kc * m)


def _col(vec):
    out = np.zeros((128, 1), np.float32)
    out[: len(vec), 0] = vec
    return out


def _pack_weights(Wm, bm, Wu, bu, W1, b1, W2, b2, W3, b3):
    f64 = np.float64
    Wm, bm, Wu, bu = Wm.astype(f64), bm.astype(f64), Wu.astype(f64), bu.astype(f64)
    W1, b1, W2, b2 = W1.astype(f64), b1.astype(f64), W2.astype(f64), b2.astype(f64)
    b3v = float(np.asarray(b3, f64).reshape(-1)[0])

    # fused z1 = xm @ Am + xu @ Au + b1p
    Am = Wm[:, :256] @ W1[:256, :]
    Au = Wu[:, :256] @ W1[256:, :]
    b1p = b1 + bm[:256] @ W1[:256, :] + bu[:256] @ W1[256:, :]

    # FM extras: p/q fold rows + additive rows
    FWm = Wm[:, :256].reshape(F, 16, 16).sum(axis=1)  # [512, 16]
    FWu = Wu[:, :256].reshape(F, 16, 16).sum(axis=1)
    fbm = bm[:256].reshape(16, 16).sum(axis=0)
    fbu = bu[:256].reshape(16, 16).sum(axis=0)
    awm, awu = Wm[:, 256], Wu[:, 256]
    A = bm[256] + bu[256] + b3v
    Xm = np.concatenate([FWm, FWm, awm[:, None] / 2, awm[:, None] / 2], axis=1)
    Xu = np.concatenate([FWu, -FWu, awu[:, None] / 2, awu[:, None] / 2], axis=1)
    xbias = np.concatenate([fbm + fbu, fbm - fbu, [(A + 1) / 2], [(A - 1) / 2]])
    wq = np.concatenate([np.full(16, 0.25), np.full(16, -0.25), [1.0, -1.0]])

    # z1 block in accumulation order: group g, then xm chunks 0-3, xu chunks 0-3
    amc, auc = _chunk(Am), _chunk(Au)  # [128, 4*256]
    z1_cols = []
    for g in range(2):
        for k in range(KC):
            z1_cols.append(amc[:, k * 256 + g * 128 : k * 256 + (g + 1) * 128])
        for k in range(KC):
            z1_cols.append(auc[:, k * 256 + g * 128 : k * 256 + (g + 1) * 128])
    w3_pad = np.zeros((128, MPAD), f64)
    w3_pad[:, 0] = np.asarray(W3, f64).reshape(128)
    wq_pad = np.zeros((128, MPAD), f64)
    wq_pad[:NX, 0] = wq
    wp = np.concatenate(
        z1_cols + [_chunk(Xm), _chunk(Xu), _chunk(W2), w3_pad, wq_pad],
        axis=1,
    )
    assert wp.shape == (128, WCOLS), wp.shape
    bp = np.concatenate(
        [_col(b1p[:128]), _col(b1p[128:]), _col(xbias), _col(b2)], axis=1
    )
    return (
        np.ascontiguousarray(wp.astype(ml_dtypes.bfloat16)),
        np.ascontiguousarray(bp.astype(np.float32)),
    )


def _build_bass():
    nc = bacc.Bacc()
    x = nc.dram_tensor("x", [128, NTILES * XT_COLS], BF16, kind="ExternalInput")
    wp = nc.dram_tensor("wp", [128, WCOLS], BF16, kind="ExternalInput")
    bp = nc.dram_tensor("bp", [128, BCOLS], F32, kind="ExternalInput")
    out = nc.dram_tensor("out", [1, R], F32, kind="ExternalOutput")

    relu = mybir.ActivationFunctionType.Relu
    square = mybir.ActivationFunctionType.Square

    with tile.TileContext(nc) as tc:
        with (
            tc.tile_pool(name="wpool", bufs=1) as wpool,
            tc.tile_pool(name="xpool", bufs=1) as xpool,
            tc.tile_pool(name="dpool", bufs=1) as dpool,
            tc.tile_pool(name="opool", bufs=1) as opool,
            tc.tile_pool(name="psz", bufs=4, space="PSUM") as psz,
            tc.tile_pool(name="psx", bufs=2, space="PSUM") as psx,
            tc.tile_pool(name="psm", bufs=1, space="PSUM") as psm,
            tc.tile_pool(name="psf", bufs=1, space="PSUM") as psf,
        ):
            # PE pre-warm matmuls on a zeroed tile (memset on the otherwise
            # idle GpSimd engine, which starts earliest), pipelined across
            # the z1 PSUM banks: covers the HAM activity window until real
            # data lands so real matmuls run at 2.4 GHz.
            wgar = wpool.tile([128, NT], BF16)
            nc.gpsimd.memset(wgar, 0.0)
            for _ in range(N_WARM):
                pw = psz.tile([128, NT], F32, name="ps_z1")
                nc.tensor.matmul(pw, wgar[:, :128], wgar, start=True, stop=True)

            # Weights on the scalar HWDGE ring (inputs get the sync ring),
            # in need-order. Each piece is its own tile so consumers wait
            # only on their own DMA.
            w0a = wpool.tile([128, Z1_COLS // 4], BF16)
            nc.scalar.dma_start(out=w0a, in_=wp[:, : Z1_COLS // 4])
            w0b = wpool.tile([128, Z1_COLS // 4], BF16)
            nc.scalar.dma_start(out=w0b, in_=wp[:, Z1_COLS // 4 : Z1_COLS // 2])
            b = wpool.tile([128, BCOLS], F32)
            nc.scalar.dma_start(out=b, in_=bp[:, :])
            w1 = wpool.tile([128, Z1_COLS // 2], BF16)
            nc.scalar.dma_start(out=w1, in_=wp[:, Z1_COLS // 2 : Z1_COLS])
            wr = wpool.tile([128, WCOLS - Z1_COLS], BF16)
            nc.scalar.dma_start(out=wr, in_=wp[:, Z1_COLS:])
            out_sb = opool.tile([128, NT], F32)

            xr = x.rearrange(
                "p (t w c n) -> p t w c n", t=NTILES, w=2, c=KC, n=NT
            )

            # Input tiles prefetch up front, one contiguous DMA per tile
            # (8KB/partition); tile 0 split finer so the first matmuls are
            # gated on as little data as possible.
            x0ma = xpool.tile([128, 2, NT], BF16)
            nc.sync.dma_start(out=x0ma, in_=xr[:, 0, 0, :2])
            x0mb = xpool.tile([128, 2, NT], BF16)
            nc.sync.dma_start(out=x0mb, in_=xr[:, 0, 0, 2:])
            x0u = xpool.tile([128, KC, NT], BF16)
            nc.sync.dma_start(out=x0u, in_=xr[:, 0, 1])
            x0m = (x0ma[:, 0], x0ma[:, 1], x0mb[:, 0], x0mb[:, 1])
            xts = [(lambda k, _m=x0m: _m[k], lambda k, _u=x0u: _u[:, k])]
            for t in range(1, NTILES):
                xt = xpool.tile([128, 2, KC, NT], BF16, name=f"x{t}")
                nc.sync.dma_start(out=xt, in_=xr[:, t])
                xts.append(
                    (lambda k, _x=xt: _x[:, 0, k], lambda k, _x=xt: _x[:, 1, k])
                )

            h1s, sqs, h2s = {}, {}, {}

            def z1_lhsT(g, j):
                if g == 0:
                    wt = w0a if j < 4 else w0b
                    return wt[:, (j % 4) * 128 : (j % 4 + 1) * 128]
                return w1[:, j * 128 : (j + 1) * 128]

            def emit_z1(t):
                xmt, xut = xts[t]
                for g in range(2):
                    ps = psz.tile([128, NT], F32, name="ps_z1")
                    for j in range(8):
                        rhs = xmt(j) if j < 4 else xut(j - 4)
                        nc.tensor.matmul(
                            ps, z1_lhsT(g, j), rhs, start=(j == 0), stop=(j == 7)
                        )
                    h = dpool.tile([128, NT], BF16, name=f"h1_{t}_{g}")
                    nc.scalar.activation(
                        out=h, in_=ps, func=relu, bias=b[:, g : g + 1]
                    )
                    h1s[(t, g)] = h

            def emit_extras(t, mid=None):
                xmt, xut = xts[t]
                ps = psx.tile([NX, NT], F32, name="ps_x")
                for j in range(8):
                    rhs = xmt(j) if j < 4 else xut(j - 4)
                    lhsT = wr[:, j * NX : (j + 1) * NX]
                    nc.tensor.matmul(
                        ps, lhsT, rhs, start=(j == 0), stop=(j == 7)
                    )
                    if j == 0 and mid is not None:
                        mid()
                sq = dpool.tile([NX, NT], BF16, name=f"sq_{t}")
                nc.scalar.activation(
                    out=sq, in_=ps, func=square, bias=b[:NX, BX : BX + 1]
                )
                sqs[t] = sq

            def emit_mlp2(t):
                ps = psm.tile([128, NT], F32, name="ps_m")
                for c in range(2):
                    lhsT = wr[:, W2_OFF - X_OFF + c * 128 : W2_OFF - X_OFF + (c + 1) * 128]
                    nc.tensor.matmul(
                        ps, lhsT, h1s[(t, c)], start=(c == 0), stop=(c == 1)
                    )
                h2 = dpool.tile([128, NT], BF16, name=f"h2_{t}")
                nc.scalar.activation(
                    out=h2, in_=ps, func=relu, bias=b[:, B2C : B2C + 1]
                )
                h2s[t] = h2

            def emit_final(t):
                ps = psf.tile([128, NT], F32, name="ps_f")
                pf = ps[FIN_POS : FIN_POS + 1]
                nc.tensor.matmul(
                    pf, wr[:, W3_OFF - X_OFF : W3_OFF - X_OFF + 1], h2s[t],
                    start=True, stop=False, tile_position=(0, FIN_POS),
                )
                nc.tensor.matmul(
                    pf, wr[:NX, WQ_OFF - X_OFF : WQ_OFF - X_OFF + 1], sqs[t],
                    start=False, stop=True, tile_position=(0, FIN_POS),
                )
                n0 = t * NT
                ob = out_sb[FIN_POS : FIN_POS + 1, :]
                nc.vector.tensor_copy(ob, ps[FIN_POS : FIN_POS + 1])
                nc.scalar.dma_start(out=out[:, n0 : n0 + NT], in_=ob)

            for t in range(NTILES):
                emit_z1(t)
                mid = (lambda tt=t: emit_final(tt - 1)) if t > 0 else None
                emit_extras(t, mid=mid)
                emit_mlp2(t)
            emit_final(NTILES - 1)
    nc.finalize()
    return nc


def _pack_x(xmT_core, xuT_core):
    """2x [512, 2048] fp32 -> [128, NTILES*2*KC*NT] bf16, tile-contiguous."""
    ym = xmT_core.reshape(KC, 128, NTILES, NT).transpose(1, 2, 0, 3)  # [p,t,c,n]
    yu = xuT_core.reshape(KC, 128, NTILES, NT).transpose(1, 2, 0, 3)
    y = np.stack([ym, yu], axis=2)  # [p, t, w, c, n]
    return np.ascontiguousarray(
        y.reshape(128, NTILES * XT_COLS).astype(ml_dtypes.bfloat16)
    )


_NC_CACHE = []


def kernel(movie_vectors, user_vectors, Wm, bm, Wu, bu, W1, b1, W2, b2, W3, b3):
    movie_vectors = np.asarray(movie_vectors, np.float32)
    user_vectors = np.asarray(user_vectors, np.float32)
    wp, bp = _pack_weights(
        np.asarray(Wm, np.float32), np.asarray(bm, np.float32),
        np.asarray(Wu, np.float32), np.asarray(bu, np.float32),
        np.asarray(W1, np.float32), np.asarray(b1, np.float32),
        np.asarray(W2, np.float32), np.asarray(b2, np.float32),
        np.asarray(W3, np.float32), np.asarray(b3, np.float32),
    )
    xmT = movie_vectors.T  # [512, 16384]
    xuT = user_vectors.T

    if not _NC_CACHE:
        _NC_CACHE.append(_build_bass())
    nc = _NC_CACHE[0]

    in_maps = []
    for c in range(N_CORES):
        sl = slice(c * R, (c + 1) * R)
        in_maps.append(
            {"x": _pack_x(xmT[:, sl], xuT[:, sl]), "wp": wp, "bp": bp}
        )
    res = run_bass_kernel_spmd(nc, in_maps, core_ids=list(range(N_CORES)))
    kernel.last_result = res
    return np.concatenate([r["out"].reshape(R, 1) for r in res.results], axis=0)


# revision 21
# speedup vs baseline: 1.1001x; 1.0613x over previous
"""DeepFM forward kernel for Trainium2, data-parallel over 8 NeuronCores.

Math refactor vs the straightforward DeepFM graph:
  1. The 256-wide tower dense outputs are only consumed by (a) the FM
     interaction sum and (b) MLP layer 1. (a) collapses to 16 "fold" sums
     per tower (sum_ij m_i.u_j = sum_k (sum_i m_ik)(sum_j u_jk)) and (b) is
     linear, so W1 is folded into the tower weights host-side:
         z1 = xm @ (Wm_d @ W1[:256]) + xu @ (Wu_d @ W1[256:]) + b1'
     The dense tower outputs are never materialized on chip.
  2. The FM sum uses the polarization identity
         sum_k fold_m.fold_u + add = sum_k (p_k^2 - q_k^2)/4 + a
     with p = fold_m + fold_u, q = fold_m - fold_u (both linear in x), and
     the additive term a riding rows 32/33 as ((a+1)/2)^2 - ((a-1)/2)^2 = a.
     The whole FM side is ONE 34-row matmul accumulation chain plus ONE
     scalar-engine Square activation, folding into the final matmul.

Everything runs in bf16 (fp32 PSUM accumulate): halves DMA traffic and
enables FWL fast weight loads. Inputs are repacked host-side so each batch
tile (both towers) is one fully-contiguous 8KB-per-partition DMA; tile 0 is
split in two so the first matmuls can start earliest. The two final M=1
matmuls are col-tiled onto PE column strip 2 (tile_position=(0,64)) so they
run concurrently with the next tile's M=34 extras chain on strips 0-1.
"""

import numpy as np
import ml_dtypes

import concourse.bacc as bacc
import concourse.bass as bass  # noqa: F401
import concourse.mybir as mybir
import concourse.tile as tile
from concourse.bass_utils import run_bass_kernel_spmd

N_CORES = 8
B_FULL = 16384
R = B_FULL // N_CORES  # 2048 rows per core
F = 512                # input features per tower
KC = F // 128          # 4 contraction chunks per tower
NT = 512               # batch tile on the free dim
NTILES = R // NT       # 4
NX = 34                # extras rows: p(16) + q(16) + a-rows(2)
N_WARM = 7             # PE pre-warm matmuls
MPAD = 33              # final matmuls padded to M=33 so they share the
                       # 128x64 col-tiling mode with the extras chain
XT_COLS = 2 * KC * NT  # per-tile input cols (both towers)

F32 = mybir.dt.float32
BF16 = mybir.dt.bfloat16

# bf16 weight-pack column offsets ([128, WCOLS] blob)
Z1_OFF = 0                      # 16 x 128: (g, j) at (g*8+j)*128; j=0-3 Am, 4-7 Au
Z1_COLS = 16 * 128
X_OFF = Z1_COLS                 # 8 x 34: j=0-3 Xm chunks, 4-7 Xu chunks
W2_OFF = X_OFF + 8 * NX         # 2 x 128
W3_OFF = W2_OFF + 2 * 128       # MPAD cols (col 0 = W3, rest zero)
WQ_OFF = W3_OFF + MPAD          # MPAD cols (col 0 rows 0-33 = fm quad weights)
WCOLS = WQ_OFF + MPAD

# fp32 bias-pack column indices ([128, BCOLS])
B1A, B1B, BX, B2C = range(4)
BCOLS = 4

FIN_POS = 64  # PE column strip for the col-tiled final matmuls


def _chunk(Wext):
    """[K, M] -> [128, (K/128)*M]: K-chunk k occupies cols [k*M, (k+1)*M)."""
    kc, m = Wext.shape[0] // 128, Wext.shape[1]
    return Wext.reshape(kc, 128, m).transpose(1, 0, 2).reshape(128, kc * m)


def _col(vec):
    out = np.zeros((128, 1), np.float32)
    out[: len(vec), 0] = vec
    return out


def _pack_weights(Wm, bm, Wu, bu, W1, b1, W2, b2, W3, b3):
    f64 = np.float64
    Wm, bm, Wu, bu = Wm.astype(f64), bm.astype(f64), Wu.astype(f64), bu.astype(f64)
    W1, b1, W2, b2 = W1.astype(f64), b1.astype(f64), W2.astype(f64), b2.astype(f64)
    b3v = float(np.asarray(b3, f64).reshape(-1)[0])

    # fused z1 = xm @ Am + xu @ Au + b1p
    Am = Wm[:, :256] @ W1[:256, :]
    Au = Wu[:, :256] @ W1[256:, :]
    b1p = b1 + bm[:256] @ W1[:256, :] + bu[:256] @ W1[256:, :]

    # FM extras: p/q fold rows + additive rows
    FWm = Wm[:, :256].reshape(F, 16, 16).sum(axis=1)  # [512, 16]
    FWu = Wu[:, :256].reshape(F, 16, 16).sum(axis=1)
    fbm = bm[:256].reshape(16, 16).sum(axis=0)
    fbu = bu[:256].reshape(16, 16).sum(axis=0)
    awm, awu = Wm[:, 256], Wu[:, 256]
    A = bm[256] + bu[256] + b3v
    Xm = np.concatenate([FWm, FWm, awm[:, None] / 2, awm[:, None] / 2], axis=1)
    Xu = np.concatenate([FWu, -FWu, awu[:, None] / 2, awu[:, None] / 2], axis=1)
    xbias = np.concatenate([fbm + fbu, fbm - fbu, [(A + 1) / 2], [(A - 1) / 2]])
    wq = np.concatenate([np.full(16, 0.25), np.full(16, -0.25), [1.0, -1.0]])

    # z1 block in accumulation order: group g, then xm chunks 0-3, xu chunks 0-3
    amc, auc = _chunk(Am), _chunk(Au)  # [128, 4*256]
    z1_cols = []
    for g in range(2):
        for k in range(KC):
            z1_cols.append(amc[:, k * 256 + g * 128 : k * 256 + (g + 1) * 128])
        for k in range(KC):
            z1_cols.append(auc[:, k * 256 + g * 128 : k * 256 + (g + 1) * 128])
    w3_pad = np.zeros((128, MPAD), f64)
    w3_pad[:, 0] = np.asarray(W3, f64).reshape(128)
    wq_pad = np.zeros((128, MPAD), f64)
    wq_pad[:NX, 0] = wq
    wp = np.concatenate(
        z1_cols + [_chunk(Xm), _chunk(Xu), _chunk(W2), w3_pad, wq_pad],
        axis=1,
    )
    assert wp.shape == (128, WCOLS), wp.shape
    bp = np.concatenate(
        [_col(b1p[:128]), _col(b1p[128:]), _col(xbias), _col(b2)], axis=1
    )
    return (
        np.ascontiguousarray(wp.astype(ml_dtypes.bfloat16)),
        np.ascontiguousarray(bp.astype(np.float32)),
    )


def _build_bass():
    nc = bacc.Bacc()
    x = nc.dram_tensor("x", [128, NTILES * XT_COLS], BF16, kind="ExternalInput")
    wp = nc.dram_tensor("wp", [128, WCOLS], BF16, kind="ExternalInput")
    bp = nc.dram_tensor("bp", [128, BCOLS], F32, kind="ExternalInput")
    out = nc.dram_tensor("out", [1, R], F32, kind="ExternalOutput")

    relu = mybir.ActivationFunctionType.Relu
    square = mybir.ActivationFunctionType.Square

    with tile.TileContext(nc) as tc:
        with (
            tc.tile_pool(name="wpool", bufs=1) as wpool,
            tc.tile_pool(name="xpool", bufs=1) as xpool,
            tc.tile_pool(name="dpool", bufs=1) as dpool,
            tc.tile_pool(name="opool", bufs=1) as opool,
            tc.tile_pool(name="psz", bufs=4, space="PSUM") as psz,
            tc.tile_pool(name="psx", bufs=2, space="PSUM") as psx,
            tc.tile_pool(name="psm", bufs=1, space="PSUM") as psm,
            tc.tile_pool(name="psf", bufs=1, space="PSUM") as psf,
        ):
            # PE pre-warm matmuls on a zeroed tile (memset on the otherwise
            # idle GpSimd engine, which starts earliest), pipelined across
            # the z1 PSUM banks: covers the HAM activity window until real
            # data lands so real matmuls run at 2.4 GHz.
            wgar = wpool.tile([128, NT], BF16)
            nc.gpsimd.memset(wgar, 0.0)
            for _ in range(N_WARM):
                pw = psz.tile([128, NT], F32, name="ps_z1")
                nc.tensor.matmul(pw, wgar[:, :128], wgar, start=True, stop=True)

            # Weights on the scalar HWDGE ring (inputs get the sync ring),
            # in need-order. Each piece is its own tile so consumers wait
            # only on their own DMA.
            # order matches PE consumption: g0-xm, g1-xm, g0-xu, g1-xu
            Q = Z1_COLS // 4
            w0a = wpool.tile([128, Q], BF16)
            nc.scalar.dma_start(out=w0a, in_=wp[:, :Q])
            w1a = wpool.tile([128, Q], BF16)
            nc.scalar.dma_start(out=w1a, in_=wp[:, 2 * Q : 3 * Q])
            w0b = wpool.tile([128, Q], BF16)
            nc.scalar.dma_start(out=w0b, in_=wp[:, Q : 2 * Q])
            w1b = wpool.tile([128, Q], BF16)
            nc.scalar.dma_start(out=w1b, in_=wp[:, 3 * Q : 4 * Q])
            wr = wpool.tile([128, WCOLS - Z1_COLS], BF16)
            nc.scalar.dma_start(out=wr, in_=wp[:, Z1_COLS:])
            b = wpool.tile([128, BCOLS], F32)
            nc.scalar.dma_start(out=b, in_=bp[:, :])
            out_sb = opool.tile([128, NT], F32)

            xr = x.rearrange(
                "p (t w c n) -> p t w c n", t=NTILES, w=2, c=KC, n=NT
            )

            # Input tiles prefetch up front, one contiguous DMA per tile
            # (8KB/partition); tile 0 split finer so the first matmuls are
            # gated on as little data as possible.
            x0ma = xpool.tile([128, 2, NT], BF16)
            nc.sync.dma_start(out=x0ma, in_=xr[:, 0, 0, :2])
            x0mb = xpool.tile([128, 2, NT], BF16)
            nc.sync.dma_start(out=x0mb, in_=xr[:, 0, 0, 2:])
            x0u = xpool.tile([128, KC, NT], BF16)
            nc.sync.dma_start(out=x0u, in_=xr[:, 0, 1])
            x0m = (x0ma[:, 0], x0ma[:, 1], x0mb[:, 0], x0mb[:, 1])
            xts = [(lambda k, _m=x0m: _m[k], lambda k, _u=x0u: _u[:, k])]
            for t in range(1, NTILES):
                xt = xpool.tile([128, 2, KC, NT], BF16, name=f"x{t}")
                nc.sync.dma_start(out=xt, in_=xr[:, t])
                xts.append(
                    (lambda k, _x=xt: _x[:, 0, k], lambda k, _x=xt: _x[:, 1, k])
                )

            h1s, sqs, h2s = {}, {}, {}

            z1w = {(0, 0): w0a, (0, 1): w0b, (1, 0): w1a, (1, 1): w1b}

            def emit_z1(t):
                # xm chunks feed BOTH output groups first, so the xu half of
                # the input DMA can land while the xm matmuls run.
                xmt, xut = xts[t]
                pss = []
                for g in range(2):
                    pss.append(psz.tile([128, NT], F32, name="ps_z1"))
                for half, xf in ((0, xmt), (1, xut)):
                    for g in range(2):
                        wt = z1w[(g, half)]
                        for k in range(KC):
                            nc.tensor.matmul(
                                pss[g], wt[:, k * 128 : (k + 1) * 128], xf(k),
                                start=(half == 0 and k == 0),
                                stop=(half == 1 and k == KC - 1),
                            )
                for g in range(2):
                    h = dpool.tile([128, NT], BF16, name=f"h1_{t}_{g}")
                    nc.scalar.activation(
                        out=h, in_=pss[g], func=relu, bias=b[:, g : g + 1]
                    )
                    h1s[(t, g)] = h

            def emit_extras(t, mid=None):
                xmt, xut = xts[t]
                ps = psx.tile([NX, NT], F32, name="ps_x")
                for j in range(8):
                    rhs = xmt(j) if j < 4 else xut(j - 4)
                    lhsT = wr[:, j * NX : (j + 1) * NX]
                    nc.tensor.matmul(
                        ps, lhsT, rhs, start=(j == 0), stop=(j == 7)
                    )
                    if j == 0 and mid is not None:
                        mid()
                sq = dpool.tile([NX, NT], BF16, name=f"sq_{t}")
                nc.scalar.activation(
                    out=sq, in_=ps, func=square, bias=b[:NX, BX : BX + 1]
                )
                sqs[t] = sq

            def emit_mlp2(t):
                ps = psm.tile([128, NT], F32, name="ps_m")
                for c in range(2):
                    lhsT = wr[:, W2_OFF - X_OFF + c * 128 : W2_OFF - X_OFF + (c + 1) * 128]
                    nc.tensor.matmul(
                        ps, lhsT, h1s[(t, c)], start=(c == 0), stop=(c == 1)
                    )
                h2 = dpool.tile([128, NT], BF16, name=f"h2_{t}")
                nc.scalar.activation(
                    out=h2, in_=ps, func=relu, bias=b[:, B2C : B2C + 1]
                )
                h2s[t] = h2

            def emit_final(t):
                ps = psf.tile([128, NT], F32, name="ps_f")
                pf = ps[FIN_POS : FIN_POS + 1]
                nc.tensor.matmul(
                    pf, wr[:, W3_OFF - X_OFF : W3_OFF - X_OFF + 1], h2s[t],
                    start=True, stop=False, tile_position=(0, FIN_POS),
                )
                nc.tensor.matmul(
                    pf, wr[:NX, WQ_OFF - X_OFF : WQ_OFF - X_OFF + 1], sqs[t],
                    start=False, stop=True, tile_position=(0, FIN_POS),
                )
                n0 = t * NT
                ob = out_sb[FIN_POS : FIN_POS + 1, :]
                nc.vector.tensor_copy(ob, ps[FIN_POS : FIN_POS + 1])
                nc.scalar.dma_start(out=out[:, n0 : n0 + NT], in_=ob)

            for t in range(NTILES):
                if t > 0:
                    emit_mlp2(t - 1)
                emit_z1(t)
                emit_extras(t)
                if t > 0:
                    emit_final(t - 1)
            emit_mlp2(NTILES - 1)
            emit_final(NTILES - 1)
    nc.finalize()
    return nc


def _pack_x(xmT_core, xuT_core):
    """2x [512, 2048] fp32 -> [128, NTILES*2*KC*NT] bf16, tile-contiguous."""
    ym = xmT_core.reshape(KC, 128, NTILES, NT).transpose(1, 2, 0, 3)  # [p,t,c,n]
    yu = xuT_core.reshape(KC, 128, NTILES, NT).transpose(1, 2, 0, 3)
    y = np.stack([ym, yu], axis=2)  # [p, t, w, c, n]
    return np.ascontiguousarray(
        y.reshape(128, NTILES * XT_COLS).astype(ml_dtypes.bfloat16)
    )


_NC_CACHE = []


def kernel(movie_vectors, user_vectors, Wm, bm, Wu, bu, W1, b1, W2, b2, W3, b3):
    movie_vectors = np.asarray(movie_vectors, np.float32)
    user_vectors = np.asarray(user_vectors, np.float32)
    wp, bp = _pack_weights(
        np.asarray(Wm, np.float32), np.asarray(bm, np.float32),
        np.asarray(Wu, np.float32), np.asarray(bu, np.float32),
        np.asarray(W1, np.float32), np.asarray(b1, np.float32),
        np.asarray(W2, np.float32), np.asarray(b2, np.float32),
        np.asarray(W3, np.float32), np.asarray(b3, np.float32),
    )
    xmT = movie_vectors.T  # [512, 16384]
    xuT = user_vectors.T

    if not _NC_CACHE:
        _NC_CACHE.append(_build_bass())
    nc = _NC_CACHE[0]

    in_maps = []
    for c in range(N_CORES):
        sl = slice(c * R, (c + 1) * R)
        in_maps.append(
            {"x": _pack_x(xmT[:, sl], xuT[:, sl]), "wp": wp, "bp": bp}
        )
    res = run_bass_kernel_spmd(nc, in_maps, core_ids=list(range(N_CORES)))
    kernel.last_result = res
    return np.concatenate([r["out"].reshape(R, 1) for r in res.results], axis=0)


# revision 26
# speedup vs baseline: 1.1158x; 1.0143x over previous
"""DeepFM forward kernel for Trainium2, data-parallel over 8 NeuronCores.

Math refactor vs the straightforward DeepFM graph:
  1. The 256-wide tower dense outputs are only consumed by (a) the FM
     interaction sum and (b) MLP layer 1. (a) collapses to 16 "fold" sums
     per tower (sum_ij m_i.u_j = sum_k (sum_i m_ik)(sum_j u_jk)) and (b) is
     linear, so W1 is folded into the tower weights host-side:
         z1 = xm @ (Wm_d @ W1[:256]) + xu @ (Wu_d @ W1[256:]) + b1'
     The dense tower outputs are never materialized on chip.
  2. The FM sum uses the polarization identity
         sum_k fold_m.fold_u + add = sum_k (p_k^2 - q_k^2)/4 + a
     with p = fold_m + fold_u, q = fold_m - fold_u (both linear in x), and
     the additive term a riding rows 32/33 as ((a+1)/2)^2 - ((a-1)/2)^2 = a.
     The whole FM side is ONE 34-row matmul accumulation chain plus ONE
     scalar-engine Square activation, folding into the final matmul.

Everything runs in bf16 (fp32 PSUM accumulate): halves DMA traffic and
enables FWL fast weight loads. Inputs are repacked host-side so each batch
tile (both towers) is one fully-contiguous 8KB-per-partition DMA; tile 0 is
split in two so the first matmuls can start earliest. The two final M=1
matmuls are col-tiled onto PE column strip 2 (tile_position=(0,64)) so they
run concurrently with the next tile's M=34 extras chain on strips 0-1.
"""

import numpy as np
import ml_dtypes

import concourse.bacc as bacc
import concourse.bass as bass  # noqa: F401
import concourse.mybir as mybir
import concourse.tile as tile
from concourse.bass_utils import run_bass_kernel_spmd

N_CORES = 8
B_FULL = 16384
R = B_FULL // N_CORES  # 2048 rows per core
F = 512                # input features per tower
KC = F // 128          # 4 contraction chunks per tower
NT = 512               # batch tile on the free dim
NTILES = R // NT       # 4
NX = 34                # extras rows: p(16) + q(16) + a-rows(2)
N_WARM = 5             # PE pre-warm matmuls
XW = 128               # extras lhsT padded to M=128: every matmul in the
                       # kernel runs in uniform 128x128 PE tiling mode, so
                       # the array never pays a mode-switch drain
XT_COLS = 2 * KC * NT  # per-tile input cols (both towers)

F32 = mybir.dt.float32
BF16 = mybir.dt.bfloat16

# bf16 weight-pack column offsets ([128, WCOLS] blob)
Z1_OFF = 0                      # 16 x 128: (g, j) at (g*8+j)*128; j=0-3 Am, 4-7 Au
Z1_COLS = 16 * 128
X_OFF = Z1_COLS                 # 8 x XW: j=0-3 Xm chunks, 4-7 Xu chunks (zero-padded)
W2_OFF = X_OFF + 8 * XW         # 2 x 128
W3_OFF = W2_OFF + 2 * 128       # 128 cols (col 0 = W3, rest zero)
WQ_OFF = W3_OFF + 128           # 128 cols (col 0 rows 0-33 = fm quad weights)
WCOLS = WQ_OFF + 128

# fp32 bias-pack column indices ([128, BCOLS])
B1A, B1B, BX, B2C = range(4)
BCOLS = 4

FIN_POS = 64  # PE column strip for the col-tiled final matmuls


def _chunk(Wext):
    """[K, M] -> [128, (K/128)*M]: K-chunk k occupies cols [k*M, (k+1)*M)."""
    kc, m = Wext.shape[0] // 128, Wext.shape[1]
    return Wext.reshape(kc, 128, m).transpose(1, 0, 2).reshape(128, kc * m)


def _col(vec):
    out = np.zeros((128, 1), np.float32)
    out[: len(vec), 0] = vec
    return out


def _pack_weights(Wm, bm, Wu, bu, W1, b1, W2, b2, W3, b3):
    f64 = np.float64
    Wm, bm, Wu, bu = Wm.astype(f64), bm.astype(f64), Wu.astype(f64), bu.astype(f64)
    W1, b1, W2, b2 = W1.astype(f64), b1.astype(f64), W2.astype(f64), b2.astype(f64)
    b3v = float(np.asarray(b3, f64).reshape(-1)[0])

    # fused z1 = xm @ Am + xu @ Au + b1p
    Am = Wm[:, :256] @ W1[:256, :]
    Au = Wu[:, :256] @ W1[256:, :]
    b1p = b1 + bm[:256] @ W1[:256, :] + bu[:256] @ W1[256:, :]

    # FM extras: p/q fold rows + additive rows
    FWm = Wm[:, :256].reshape(F, 16, 16).sum(axis=1)  # [512, 16]
    FWu = Wu[:, :256].reshape(F, 16, 16).sum(axis=1)
    fbm = bm[:256].reshape(16, 16).sum(axis=0)
    fbu = bu[:256].reshape(16, 16).sum(axis=0)
    awm, awu = Wm[:, 256], Wu[:, 256]
    A = bm[256] + bu[256] + b3v
    Xm = np.concatenate([FWm, FWm, awm[:, None] / 2, awm[:, None] / 2], axis=1)
    Xu = np.concatenate([FWu, -FWu, awu[:, None] / 2, awu[:, None] / 2], axis=1)
    xbias = np.concatenate([fbm + fbu, fbm - fbu, [(A + 1) / 2], [(A - 1) / 2]])
    wq = np.concatenate([np.full(16, 0.25), np.full(16, -0.25), [1.0, -1.0]])

    # z1 block in accumulation order: group g, then xm chunks 0-3, xu chunks 0-3
    amc, auc = _chunk(Am), _chunk(Au)  # [128, 4*256]
    z1_cols = []
    for g in range(2):
        for k in range(KC):
            z1_cols.append(amc[:, k * 256 + g * 128 : k * 256 + (g + 1) * 128])
        for k in range(KC):
            z1_cols.append(auc[:, k * 256 + g * 128 : k * 256 + (g + 1) * 128])
    Xm_pad = np.zeros((F, XW), f64)
    Xm_pad[:, :NX] = Xm
    Xu_pad = np.zeros((F, XW), f64)
    Xu_pad[:, :NX] = Xu
    w3_pad = np.zeros((128, 128), f64)
    w3_pad[:, 0] = np.asarray(W3, f64).reshape(128)
    wq_pad = np.zeros((128, 128), f64)
    wq_pad[:NX, 0] = wq
    wp = np.concatenate(
        z1_cols + [_chunk(Xm_pad), _chunk(Xu_pad), _chunk(W2), w3_pad, wq_pad],
        axis=1,
    )
    assert wp.shape == (128, WCOLS), wp.shape
    bp = np.concatenate(
        [_col(b1p[:128]), _col(b1p[128:]), _col(xbias), _col(b2)], axis=1
    )
    return (
        np.ascontiguousarray(wp.astype(ml_dtypes.bfloat16)),
        np.ascontiguousarray(bp.astype(np.float32)),
    )


def _build_bass():
    nc = bacc.Bacc()
    x = nc.dram_tensor("x", [128, NTILES * XT_COLS], BF16, kind="ExternalInput")
    wp = nc.dram_tensor("wp", [128, WCOLS], BF16, kind="ExternalInput")
    bp = nc.dram_tensor("bp", [128, BCOLS], F32, kind="ExternalInput")
    out = nc.dram_tensor("out", [1, R], F32, kind="ExternalOutput")

    relu = mybir.ActivationFunctionType.Relu
    square = mybir.ActivationFunctionType.Square

    with tile.TileContext(nc) as tc:
        with (
            tc.tile_pool(name="wpool", bufs=1) as wpool,
            tc.tile_pool(name="xpool", bufs=1) as xpool,
            tc.tile_pool(name="dpool", bufs=1) as dpool,
            tc.tile_pool(name="opool", bufs=1) as opool,
            tc.tile_pool(name="psz", bufs=4, space="PSUM") as psz,
            tc.tile_pool(name="psx", bufs=2, space="PSUM") as psx,
            tc.tile_pool(name="psm", bufs=1, space="PSUM") as psm,
            tc.tile_pool(name="psf", bufs=1, space="PSUM") as psf,
        ):
            # PE pre-warm matmuls on a zeroed tile (memset on the otherwise
            # idle GpSimd engine, which starts earliest), pipelined across
            # the z1 PSUM banks: covers the HAM activity window until real
            # data lands so real matmuls run at 2.4 GHz.
            wgar = wpool.tile([128, NT], BF16)
            nc.gpsimd.memset(wgar, 0.0)
            for _ in range(N_WARM):
                pw = psz.tile([128, NT], F32, name="ps_z1")
                nc.tensor.matmul(pw, wgar[:, :128], wgar, start=True, stop=True)

            # Weights on the scalar HWDGE ring (inputs get the sync ring),
            # in need-order. Each piece is its own tile so consumers wait
            # only on their own DMA.
            # order matches PE consumption: g0-xm, g1-xm, g0-xu, g1-xu
            Q = Z1_COLS // 4
            w0a = wpool.tile([128, Q], BF16)
            nc.scalar.dma_start(out=w0a, in_=wp[:, :Q])
            w1a = wpool.tile([128, Q], BF16)
            nc.scalar.dma_start(out=w1a, in_=wp[:, 2 * Q : 3 * Q])
            w0b = wpool.tile([128, Q], BF16)
            nc.scalar.dma_start(out=w0b, in_=wp[:, Q : 2 * Q])
            w1b = wpool.tile([128, Q], BF16)
            nc.scalar.dma_start(out=w1b, in_=wp[:, 3 * Q : 4 * Q])
            wr = wpool.tile([128, WCOLS - Z1_COLS], BF16)
            nc.scalar.dma_start(out=wr, in_=wp[:, Z1_COLS:])
            b = wpool.tile([128, BCOLS], F32)
            nc.scalar.dma_start(out=b, in_=bp[:, :])
            out_sb = opool.tile([128, NT], F32)

            xr = x.rearrange(
                "p (t w c n) -> p t w c n", t=NTILES, w=2, c=KC, n=NT
            )

            # Input tiles prefetch up front, one contiguous DMA per tile
            # (8KB/partition); tile 0 split finer so the first matmuls are
            # gated on as little data as possible.
            x0ma = xpool.tile([128, 2, NT], BF16)
            nc.sync.dma_start(out=x0ma, in_=xr[:, 0, 0, :2])
            x0mb = xpool.tile([128, 2, NT], BF16)
            nc.sync.dma_start(out=x0mb, in_=xr[:, 0, 0, 2:])
            x0u = xpool.tile([128, KC, NT], BF16)
            nc.sync.dma_start(out=x0u, in_=xr[:, 0, 1])
            x0m = (x0ma[:, 0], x0ma[:, 1], x0mb[:, 0], x0mb[:, 1])
            xts = [(lambda k, _m=x0m: _m[k], lambda k, _u=x0u: _u[:, k])]
            for t in range(1, NTILES):
                xt = xpool.tile([128, 2, KC, NT], BF16, name=f"x{t}")
                nc.sync.dma_start(out=xt, in_=xr[:, t])
                xts.append(
                    (lambda k, _x=xt: _x[:, 0, k], lambda k, _x=xt: _x[:, 1, k])
                )

            h1s, sqs, h2s = {}, {}, {}

            z1w = {(0, 0): w0a, (0, 1): w0b, (1, 0): w1a, (1, 1): w1b}

            def emit_z1(t):
                # xm chunks feed BOTH output groups first, so the xu half of
                # the input DMA can land while the xm matmuls run.
                xmt, xut = xts[t]
                pss = []
                for g in range(2):
                    pss.append(psz.tile([128, NT], F32, name="ps_z1"))
                for half, xf in ((0, xmt), (1, xut)):
                    for g in range(2):
                        wt = z1w[(g, half)]
                        for k in range(KC):
                            nc.tensor.matmul(
                                pss[g], wt[:, k * 128 : (k + 1) * 128], xf(k),
                                start=(half == 0 and k == 0),
                                stop=(half == 1 and k == KC - 1),
                            )
                for g in range(2):
                    h = dpool.tile([128, NT], BF16, name=f"h1_{t}_{g}")
                    nc.scalar.activation(
                        out=h, in_=pss[g], func=relu, bias=b[:, g : g + 1]
                    )
                    h1s[(t, g)] = h

            def emit_extras(t):
                # M padded to 128 (rows 34-127 have zero weights, so PSUM
                # rows 34-127 are written as zeros and sq is fully defined).
                xmt, xut = xts[t]
                ps = psx.tile([128, NT], F32, name="ps_x")
                for j in range(8):
                    rhs = xmt(j) if j < 4 else xut(j - 4)
                    lhsT = wr[:, j * XW : (j + 1) * XW]
                    nc.tensor.matmul(
                        ps, lhsT, rhs, start=(j == 0), stop=(j == 7)
                    )
                sq = dpool.tile([128, NT], BF16, name=f"sq_{t}")
                nc.scalar.activation(
                    out=sq, in_=ps, func=square, bias=b[:, BX : BX + 1]
                )
                sqs[t] = sq

            def emit_mlp2(t):
                ps = psm.tile([128, NT], F32, name="ps_m")
                for c in range(2):
                    lhsT = wr[:, W2_OFF - X_OFF + c * 128 : W2_OFF - X_OFF + (c + 1) * 128]
                    nc.tensor.matmul(
                        ps, lhsT, h1s[(t, c)], start=(c == 0), stop=(c == 1)
                    )
                h2 = dpool.tile([128, NT], BF16, name=f"h2_{t}")
                nc.scalar.activation(
                    out=h2, in_=ps, func=relu, bias=b[:, B2C : B2C + 1]
                )
                h2s[t] = h2

            def emit_final(t):
                # K=128 x M=128 zero-padded: row 0 of psf carries the logit;
                # stays in the uniform full-array tiling mode.
                ps = psf.tile([128, NT], F32, name="ps_f")
                nc.tensor.matmul(
                    ps, wr[:, W3_OFF - X_OFF : W3_OFF - X_OFF + 128], h2s[t],
                    start=True, stop=False,
                )
                nc.tensor.matmul(
                    ps, wr[:, WQ_OFF - X_OFF : WQ_OFF - X_OFF + 128], sqs[t],
                    start=False, stop=True,
                )
                n0 = t * NT
                ob = out_sb[0:1, :]
                nc.vector.tensor_copy(ob, ps[0:1])
                nc.scalar.dma_start(out=out[:, n0 : n0 + NT], in_=ob)

            for t in range(NTILES):
                if t > 0:
                    emit_mlp2(t - 1)
                emit_z1(t)
                emit_extras(t)
                if t > 0:
                    emit_final(t - 1)
            emit_mlp2(NTILES - 1)
            emit_final(NTILES - 1)
    nc.finalize()
    return nc


def _pack_x(xmT_core, xuT_core):
    """2x [512, 2048] fp32 -> [128, NTILES*2*KC*NT] bf16, tile-contiguous."""
    ym = xmT_core.reshape(KC, 128, NTILES, NT).transpose(1, 2, 0, 3)  # [p,t,c,n]
    yu = xuT_core.reshape(KC, 128, NTILES, NT).transpose(1, 2, 0, 3)
    y = np.stack([ym, yu], axis=2)  # [p, t, w, c, n]
    return np.ascontiguousarray(
        y.reshape(128, NTILES * XT_COLS).astype(ml_dtypes.bfloat16)
    )


_NC_CACHE = []


def kernel(movie_vectors, user_vectors, Wm, bm, Wu, bu, W1, b1, W2, b2, W3, b3):
    movie_vectors = np.asarray(movie_vectors, np.float32)
    user_vectors = np.asarray(user_vectors, np.float32)
    wp, bp = _pack_weights(
        np.asarray(Wm, np.float32), np.asarray(bm, np.float32),
        np.asarray(Wu, np.float32), np.asarray(bu, np.float32),
        np.asarray(W1, np.float32), np.asarray(b1, np.float32),
        np.asarray(W2, np.float32), np.asarray(b2, np.float32),
        np.asarray(W3, np.float32), np.asarray(b3, np.float32),
    )
    xmT = movie_vectors.T  # [512, 16384]
    xuT = user_vectors.T

    if not _NC_CACHE:
        _NC_CACHE.append(_build_bass())
    nc = _NC_CACHE[0]

    in_maps = []
    for c in range(N_CORES):
        sl = slice(c * R, (c + 1) * R)
        in_maps.append(
            {"x": _pack_x(xmT[:, sl], xuT[:, sl]), "wp": wp, "bp": bp}
        )
    res = run_bass_kernel_spmd(nc, in_maps, core_ids=list(range(N_CORES)))
    kernel.last_result = res
    return np.concatenate([r["out"].reshape(R, 1) for r in res.results], axis=0)


# revision 31
# speedup vs baseline: 1.1165x; 1.0006x over previous
"""DeepFM forward kernel for Trainium2, data-parallel over 8 NeuronCores.

Math refactor vs the straightforward DeepFM graph:
  1. The 256-wide tower dense outputs are only consumed by (a) the FM
     interaction sum and (b) MLP layer 1. (a) collapses to 16 "fold" sums
     per tower (sum_ij m_i.u_j = sum_k (sum_i m_ik)(sum_j u_jk)) and (b) is
     linear, so W1 is folded into the tower weights host-side:
         z1 = xm @ (Wm_d @ W1[:256]) + xu @ (Wu_d @ W1[256:]) + b1'
     The dense tower outputs are never materialized on chip.
  2. The FM sum uses the polarization identity
         sum_k fold_m.fold_u + add = sum_k (p_k^2 - q_k^2)/4 + a
     with p = fold_m + fold_u, q = fold_m - fold_u (both linear in x), and
     the additive term a riding rows 32/33 as ((a+1)/2)^2 - ((a-1)/2)^2 = a.
     The whole FM side is ONE 34-row matmul accumulation chain plus ONE
     scalar-engine Square activation, folding into the final matmul.

Everything runs in bf16 (fp32 PSUM accumulate): halves DMA traffic and
enables FWL fast weight loads. Inputs are repacked host-side so each batch
tile (both towers) is one fully-contiguous 8KB-per-partition DMA; tile 0 is
split in two so the first matmuls can start earliest. The two final M=1
matmuls are col-tiled onto PE column strip 2 (tile_position=(0,64)) so they
run concurrently with the next tile's M=34 extras chain on strips 0-1.
"""

import numpy as np
import ml_dtypes

import concourse.bacc as bacc
import concourse.bass as bass  # noqa: F401
import concourse.mybir as mybir
import concourse.tile as tile
from concourse.bass_utils import run_bass_kernel_spmd

N_CORES = 8
B_FULL = 16384
R = B_FULL // N_CORES  # 2048 rows per core
F = 512                # input features per tower
KC = F // 128          # 4 contraction chunks per tower
NT = 512               # batch tile on the free dim
NTILES = R // NT       # 4
NX = 34                # extras rows: p(16) + q(16) + a-rows(2)
N_WARM = 5             # PE pre-warm matmuls
XW = 128               # extras lhsT padded to M=128: every matmul in the
                       # kernel runs in uniform 128x128 PE tiling mode, so
                       # the array never pays a mode-switch drain
XT_COLS = 2 * KC * NT  # per-tile input cols (both towers)

F32 = mybir.dt.float32
BF16 = mybir.dt.bfloat16

# bf16 weight-pack column offsets ([128, WCOLS] blob)
Z1_OFF = 0                      # 16 x 128: (g, j) at (g*8+j)*128; j=0-3 Am, 4-7 Au
Z1_COLS = 16 * 128
X_OFF = Z1_COLS                 # 8 x XW: j=0-3 Xm chunks, 4-7 Xu chunks (zero-padded)
W2_OFF = X_OFF + 8 * XW         # 2 x 128
W3_OFF = W2_OFF + 2 * 128       # 128 cols (col 0 = W3, rest zero)
WQ_OFF = W3_OFF + 128           # 128 cols (col 0 rows 0-33 = fm quad weights)
WCOLS = WQ_OFF + 128

# fp32 bias-pack column indices ([128, BCOLS])
B1A, B1B, BX, B2C = range(4)
BCOLS = 4

FIN_POS = 64  # PE column strip for the col-tiled final matmuls


def _chunk(Wext):
    """[K, M] -> [128, (K/128)*M]: K-chunk k occupies cols [k*M, (k+1)*M)."""
    kc, m = Wext.shape[0] // 128, Wext.shape[1]
    return Wext.reshape(kc, 128, m).transpose(1, 0, 2).reshape(128, kc * m)


def _col(vec):
    out = np.zeros((128, 1), np.float32)
    out[: len(vec), 0] = vec
    return out


def _pack_weights(Wm, bm, Wu, bu, W1, b1, W2, b2, W3, b3):
    f64 = np.float64
    Wm, bm, Wu, bu = Wm.astype(f64), bm.astype(f64), Wu.astype(f64), bu.astype(f64)
    W1, b1, W2, b2 = W1.astype(f64), b1.astype(f64), W2.astype(f64), b2.astype(f64)
    b3v = float(np.asarray(b3, f64).reshape(-1)[0])

    # fused z1 = xm @ Am + xu @ Au + b1p
    Am = Wm[:, :256] @ W1[:256, :]
    Au = Wu[:, :256] @ W1[256:, :]
    b1p = b1 + bm[:256] @ W1[:256, :] + bu[:256] @ W1[256:, :]

    # FM extras: p/q fold rows + additive rows
    FWm = Wm[:, :256].reshape(F, 16, 16).sum(axis=1)  # [512, 16]
    FWu = Wu[:, :256].reshape(F, 16, 16).sum(axis=1)
    fbm = bm[:256].reshape(16, 16).sum(axis=0)
    fbu = bu[:256].reshape(16, 16).sum(axis=0)
    awm, awu = Wm[:, 256], Wu[:, 256]
    A = bm[256] + bu[256] + b3v
    Xm = np.concatenate([FWm, FWm, awm[:, None] / 2, awm[:, None] / 2], axis=1)
    Xu = np.concatenate([FWu, -FWu, awu[:, None] / 2, awu[:, None] / 2], axis=1)
    xbias = np.concatenate([fbm + fbu, fbm - fbu, [(A + 1) / 2], [(A - 1) / 2]])
    wq = np.concatenate([np.full(16, 0.25), np.full(16, -0.25), [1.0, -1.0]])

    # z1 block in accumulation order: group g, then xm chunks 0-3, xu chunks 0-3
    amc, auc = _chunk(Am), _chunk(Au)  # [128, 4*256]
    z1_cols = []
    for g in range(2):
        for k in range(KC):
            z1_cols.append(amc[:, k * 256 + g * 128 : k * 256 + (g + 1) * 128])
        for k in range(KC):
            z1_cols.append(auc[:, k * 256 + g * 128 : k * 256 + (g + 1) * 128])
    Xm_pad = np.zeros((F, XW), f64)
    Xm_pad[:, :NX] = Xm
    Xu_pad = np.zeros((F, XW), f64)
    Xu_pad[:, :NX] = Xu
    w3_pad = np.zeros((128, 128), f64)
    w3_pad[:, 0] = np.asarray(W3, f64).reshape(128)
    wq_pad = np.zeros((128, 128), f64)
    wq_pad[:NX, 0] = wq
    wp = np.concatenate(
        z1_cols + [_chunk(Xm_pad), _chunk(Xu_pad), _chunk(W2), w3_pad, wq_pad],
        axis=1,
    )
    assert wp.shape == (128, WCOLS), wp.shape
    bp = np.concatenate(
        [_col(b1p[:128]), _col(b1p[128:]), _col(xbias), _col(b2)], axis=1
    )
    return (
        np.ascontiguousarray(wp.astype(ml_dtypes.bfloat16)),
        np.ascontiguousarray(bp.astype(np.float32)),
    )


def _build_bass():
    nc = bacc.Bacc()
    x = nc.dram_tensor("x", [128, NTILES * XT_COLS], BF16, kind="ExternalInput")
    wp = nc.dram_tensor("wp", [128, WCOLS], BF16, kind="ExternalInput")
    bp = nc.dram_tensor("bp", [128, BCOLS], F32, kind="ExternalInput")
    out = nc.dram_tensor("out", [1, R], F32, kind="ExternalOutput")

    relu = mybir.ActivationFunctionType.Relu
    square = mybir.ActivationFunctionType.Square

    with tile.TileContext(nc) as tc:
        with (
            tc.tile_pool(name="wpool", bufs=1) as wpool,
            tc.tile_pool(name="xpool", bufs=1) as xpool,
            tc.tile_pool(name="dpool", bufs=1) as dpool,
            tc.tile_pool(name="opool", bufs=1) as opool,
            tc.tile_pool(name="psz", bufs=3, space="PSUM") as psz,
            tc.tile_pool(name="psx", bufs=2, space="PSUM") as psx,
            tc.tile_pool(name="psm", bufs=1, space="PSUM") as psm,
            tc.tile_pool(name="psf", bufs=2, space="PSUM") as psf,
        ):
            # PE pre-warm matmuls on a zeroed tile (memset on the otherwise
            # idle GpSimd engine, which starts earliest), pipelined across
            # the z1 PSUM banks: covers the HAM activity window until real
            # data lands so real matmuls run at 2.4 GHz.
            wgar = wpool.tile([128, NT], BF16)
            nc.gpsimd.memset(wgar, 0.0)
            for _ in range(N_WARM):
                pw = psz.tile([128, NT], F32, name="ps_z1")
                nc.tensor.matmul(pw, wgar[:, :128], wgar, start=True, stop=True)

            # Weights on the scalar HWDGE ring (inputs get the sync ring),
            # in need-order. Each piece is its own tile so consumers wait
            # only on their own DMA.
            # order matches PE consumption: g0-xm, g1-xm, g0-xu, g1-xu
            Q = Z1_COLS // 4
            w0a = wpool.tile([128, Q], BF16)
            nc.scalar.dma_start(out=w0a, in_=wp[:, :Q])
            w1a = wpool.tile([128, Q], BF16)
            nc.scalar.dma_start(out=w1a, in_=wp[:, 2 * Q : 3 * Q])
            w0b = wpool.tile([128, Q], BF16)
            nc.scalar.dma_start(out=w0b, in_=wp[:, Q : 2 * Q])
            w1b = wpool.tile([128, Q], BF16)
            nc.scalar.dma_start(out=w1b, in_=wp[:, 3 * Q : 4 * Q])
            b = wpool.tile([128, BCOLS], F32)
            nc.scalar.dma_start(out=b, in_=bp[:, :])
            wr = wpool.tile([128, WCOLS - Z1_COLS], BF16)
            nc.scalar.dma_start(out=wr, in_=wp[:, Z1_COLS:])
            out_sb = opool.tile([128, NT], F32)

            xr = x.rearrange(
                "p (t w c n) -> p t w c n", t=NTILES, w=2, c=KC, n=NT
            )

            # Input tiles prefetch up front, one contiguous DMA per tile
            # (8KB/partition); tile 0 split finer so the first matmuls are
            # gated on as little data as possible.
            x0ma = xpool.tile([128, 2, NT], BF16)
            nc.sync.dma_start(out=x0ma, in_=xr[:, 0, 0, :2])
            x0mb = xpool.tile([128, 2, NT], BF16)
            nc.sync.dma_start(out=x0mb, in_=xr[:, 0, 0, 2:])
            x0ua = xpool.tile([128, 2, NT], BF16)
            nc.sync.dma_start(out=x0ua, in_=xr[:, 0, 1, :2])
            x0ub = xpool.tile([128, 2, NT], BF16)
            nc.sync.dma_start(out=x0ub, in_=xr[:, 0, 1, 2:])
            x0m = (x0ma[:, 0], x0ma[:, 1], x0mb[:, 0], x0mb[:, 1])
            x0u = (x0ua[:, 0], x0ua[:, 1], x0ub[:, 0], x0ub[:, 1])
            xts = [(lambda k, _m=x0m: _m[k], lambda k, _u=x0u: _u[k])]
            for t in range(1, NTILES):
                xt = xpool.tile([128, 2, KC, NT], BF16, name=f"x{t}")
                nc.sync.dma_start(out=xt, in_=xr[:, t])
                xts.append(
                    (lambda k, _x=xt: _x[:, 0, k], lambda k, _x=xt: _x[:, 1, k])
                )

            h1s, sqs, h2s = {}, {}, {}

            z1w = {(0, 0): w0a, (0, 1): w0b, (1, 0): w1a, (1, 1): w1b}

            def emit_z1(t):
                # xm chunks feed BOTH output groups first, so the xu half of
                # the input DMA can land while the xm matmuls run. Each
                # group's relu is emitted right after that group's stop
                # matmul so its semaphore wait isn't merged to the z1 end.
                xmt, xut = xts[t]
                pss = []
                for g in range(2):
                    pss.append(psz.tile([128, NT], F32, name="ps_z1"))
                for half, xf in ((0, xmt), (1, xut)):
                    for g in range(2):
                        wt = z1w[(g, half)]
                        for k in range(KC):
                            nc.tensor.matmul(
                                pss[g], wt[:, k * 128 : (k + 1) * 128], xf(k),
                                start=(half == 0 and k == 0),
                                stop=(half == 1 and k == KC - 1),
                            )
                        if half == 1:
                            h = dpool.tile([128, NT], BF16, name=f"h1_{t}_{g}")
                            nc.scalar.activation(
                                out=h, in_=pss[g], func=relu, bias=b[:, g : g + 1]
                            )
                            h1s[(t, g)] = h

            def emit_extras(t):
                # M padded to 128 (rows 34-127 have zero weights, so PSUM
                # rows 34-127 are written as zeros and sq is fully defined).
                xmt, xut = xts[t]
                ps = psx.tile([128, NT], F32, name="ps_x")
                for j in range(8):
                    rhs = xmt(j) if j < 4 else xut(j - 4)
                    lhsT = wr[:, j * XW : (j + 1) * XW]
                    nc.tensor.matmul(
                        ps, lhsT, rhs, start=(j == 0), stop=(j == 7)
                    )
                sq = dpool.tile([128, NT], BF16, name=f"sq_{t}")
                nc.scalar.activation(
                    out=sq, in_=ps, func=square, bias=b[:, BX : BX + 1]
                )
                sqs[t] = sq

            def emit_mlp2(t):
                ps = psm.tile([128, NT], F32, name="ps_m")
                for c in range(2):
                    lhsT = wr[:, W2_OFF - X_OFF + c * 128 : W2_OFF - X_OFF + (c + 1) * 128]
                    nc.tensor.matmul(
                        ps, lhsT, h1s[(t, c)], start=(c == 0), stop=(c == 1)
                    )
                h2 = dpool.tile([128, NT], BF16, name=f"h2_{t}")
                nc.scalar.activation(
                    out=h2, in_=ps, func=relu, bias=b[:, B2C : B2C + 1]
                )
                h2s[t] = h2

            def emit_final(t):
                # K=128 x M=128 zero-padded: row 0 of psf carries the logit;
                # stays in the uniform full-array tiling mode.
                ps = psf.tile([128, NT], F32, name="ps_f")
                nc.tensor.matmul(
                    ps, wr[:, W3_OFF - X_OFF : W3_OFF - X_OFF + 128], h2s[t],
                    start=True, stop=False,
                )
                nc.tensor.matmul(
                    ps, wr[:, WQ_OFF - X_OFF : WQ_OFF - X_OFF + 128], sqs[t],
                    start=False, stop=True,
                )
                n0 = t * NT
                ob = out_sb[0:1, :]
                nc.vector.tensor_copy(ob, ps[0:1])
                nc.scalar.dma_start(out=out[:, n0 : n0 + NT], in_=ob)

            for t in range(NTILES):
                if t > 0:
                    emit_mlp2(t - 1)
                emit_z1(t)
                emit_extras(t)
                if 0 < t < NTILES - 1:
                    emit_final(t - 1)
            emit_mlp2(NTILES - 1)
            # fin_2 here covers the h2 relu latency before fin_3
            emit_final(NTILES - 2)
            emit_final(NTILES - 1)
    nc.finalize()
    return nc


def _pack_x(xmT_core, xuT_core):
    """2x [512, 2048] fp32 -> [128, NTILES*2*KC*NT] bf16, tile-contiguous."""
    ym = xmT_core.reshape(KC, 128, NTILES, NT).transpose(1, 2, 0, 3)  # [p,t,c,n]
    yu = xuT_core.reshape(KC, 128, NTILES, NT).transpose(1, 2, 0, 3)
    y = np.stack([ym, yu], axis=2)  # [p, t, w, c, n]
    return np.ascontiguousarray(
        y.reshape(128, NTILES * XT_COLS).astype(ml_dtypes.bfloat16)
    )


_NC_CACHE = []


def kernel(movie_vectors, user_vectors, Wm, bm, Wu, bu, W1, b1, W2, b2, W3, b3):
    movie_vectors = np.asarray(movie_vectors, np.float32)
    user_vectors = np.asarray(user_vectors, np.float32)
    wp, bp = _pack_weights(
        np.asarray(Wm, np.float32), np.asarray(bm, np.float32),
        np.asarray(Wu, np.float32), np.asarray(bu, np.float32),
        np.asarray(W1, np.float32), np.asarray(b1, np.float32),
        np.asarray(W2, np.float32), np.asarray(b2, np.float32),
        np.asarray(W3, np.float32), np.asarray(b3, np.float32),
    )
    xmT = movie_vectors.T  # [512, 16384]
    xuT = user_vectors.T

    if not _NC_CACHE:
        _NC_CACHE.append(_build_bass())
    nc = _NC_CACHE[0]

    in_maps = []
    for c in range(N_CORES):
        sl = slice(c * R, (c + 1) * R)
        in_maps.append(
            {"x": _pack_x(xmT[:, sl], xuT[:, sl]), "wp": wp, "bp": bp}
        )
    res = run_bass_kernel_spmd(nc, in_maps, core_ids=list(range(N_CORES)))
    kernel.last_result = res
    return np.concatenate([r["out"].reshape(R, 1) for r in res.results], axis=0)
